# revision 6
# baseline (speedup 1.0000x reference)
"""Bidirectional tanh-RNN (B=32, S=512, I=H=1024) on 8 Trainium2 NeuronCores.

Sharding: 2 direction groups x 4 cores (cores 0-3 fwd, 4-7 bwd; host
reverses time for bwd). Batch split 4 ways -> BL=8 sequences per core.

Per-core kernel v2 (bf16 recurrence, PE column-tiled):
  - Recurrence h @ W_hh.T runs as 64 matmuls/step: 4 PE column-groups
    (tile_position=(0,32g)) x 8 k-tiles x 2 N-halves of 128. The four
    column groups stream concurrently -> ~2048 PE cycles/step instead of
    8192. Stationary = hT slices [128,8] bf16 (tiny loads); moving =
    W_hhT rows (resident bf16, host-permuted so that the stream-transpose
    block layout of h is consumed directly).
  - PSUM layout: one [128,256] tile per step parity; column-group g owns
    partitions 32g..32g+7, holding h columns 256g..256g+255.
  - tanh: 2 ACT instrs ([128,0:128], [128,128:256]) -> H_sb bf16.
  - hT: 2 DVE 32x32 stream-transposes ([128,128] each) -> T_sb; block
    (a,b_) of T holds h[j, 256a+32b_+i] at [32a+i, 32b_+j], so the
    k-tile-k stationary is simply T[:, 32k:32k+8] given the u row
    permutation kappa(p,k) = 256*(p//32) + 32k + p%32.
  - xp = x @ W_ih.T + bias precomputed in 16-step chunks (f32r, N=512
    matmuls through 2 PSUM banks), staged to strip-layout SBUF tiles via
    one SBUF->SBUF DMA per chunk, DVE-copied [128,256] into the parity
    PSUM one step ahead (has_written bits persist from prologue dummies).
  - Output: h stored bf16, out[8t+b, n] = h_t[b, n]; host converts.
"""

import numpy as np
from contextlib import ExitStack

import concourse.bass as bass
import concourse.mybir as mybir
import concourse.tile as tile
from concourse import bacc

F32 = mybir.dt.float32
F32R = mybir.dt.float32r
BF16 = mybir.dt.bfloat16

B, S, I, H = 32, 512, 1024, 1024
NCORES = 8
BL = 8          # local batch per core
KT = 8          # 128-row contraction tiles over I/H
CH = 16         # steps per projection chunk (M-tile of 128 = 16*8 rows)
NG = 4          # PE column groups
CW = 256        # h columns per group


def _emit_body(ctx: ExitStack, tc: tile.TileContext, xT, w, u, bias, out, steps):
    nc = tc.nc
    n_chunks = steps // CH
    assert steps % CH == 0

    const = ctx.enter_context(tc.tile_pool(name="const", bufs=1))
    xpool = ctx.enter_context(tc.tile_pool(name="xc", bufs=2))
    ppool = ctx.enter_context(tc.tile_pool(name="proj", bufs=1, space="PSUM"))
    rpool = ctx.enter_context(tc.tile_pool(name="rec", bufs=1, space="PSUM"))
    pspool = ctx.enter_context(tc.tile_pool(name="pjs", bufs=2))
    hpool = ctx.enter_context(tc.tile_pool(name="h", bufs=3))
    tpool = ctx.enter_context(tc.tile_pool(name="t", bufs=3))
    dpool = ctx.enter_context(tc.tile_pool(name="dram", bufs=1, space="DRAM"))

    # --- constants / resident weights ---
    w_sb = const.tile([128, KT, H], F32R)
    nc.gpsimd.dma_start(w_sb[:], w.rearrange("(k p) n -> p k n", p=128))
    u_sb = const.tile([128, KT, H], BF16)
    nc.gpsimd.dma_start(u_sb[:], u.rearrange("p (k n) -> p k n", k=KT))
    bias_sb = const.tile([1, H], F32R)
    nc.gpsimd.dma_start(bias_sb[:], bias[:])
    ones1_f = const.tile([1, 128], F32)
    nc.gpsimd.memset(ones1_f[:], 1.0)
    ones1 = const.tile([1, 128], F32R)
    nc.vector.tensor_copy(ones1[:], ones1_f[:])

    # strip-layout xp staging tiles (persistent; garbage rows zeroed once)
    X_bufs = [const.tile([128, CH, CW], F32, name=f"Xb{i}") for i in range(2)]
    for Xb in X_bufs:
        nc.gpsimd.memset(Xb[:], 0.0)

    xT_r = xT.rearrange("(k p) m -> p k m", p=128)
    xp_dram = dpool.tile([steps * BL, H], F32)

    from collections import deque
    proj_pending = deque()

    def queue_proj(c):
        """Queue chunk c's projection matmuls as bubble-filler thunks."""
        xc = xpool.tile([128, KT, CH * BL], F32R, tag="xc")
        nc.gpsimd.dma_start(xc[:], xT_r[:, :, c * CH * BL:(c + 1) * CH * BL])
        xs_sb = pspool.tile([128, H], F32, tag="pjs")
        state = {}

        def mk(b2, k):
            nbs = slice(512 * b2, 512 * (b2 + 1))

            def run():
                if k == -1:
                    p = ppool.tile([128, 512], F32, tag=f"pj{b2}")
                    state[b2] = p
                    nc.tensor.matmul(p[:], lhsT=ones1[:],
                                     rhs=bias_sb[:, nbs],
                                     start=True, stop=False)
                    return
                p = state[b2]
                nc.tensor.matmul(p[:], lhsT=xc[:, k, :],
                                 rhs=w_sb[:, k, nbs],
                                 start=False, stop=(k == KT - 1))
                if k == KT - 1:
                    # bank 0 copy on DVE, bank 1 on ACT: halves the per-chunk
                    # copy spike queued in front of chain-critical tanh (ACT)
                    # and transpose (DVE).
                    if b2 == 0:
                        nc.vector.tensor_copy(xs_sb[:, nbs], p[:])
                    else:
                        nc.scalar.activation(xs_sb[:, nbs], p[:],
                                             mybir.ActivationFunctionType.Copy)
                    if b2 == 1:
                        # stage to DRAM, then strip-relayout back:
                        # X[32g+b, s, c] <- xp_dram[128c + 8s + b, 256g + c]
                        xd = xp_dram[128 * c:128 * (c + 1), :]
                        nc.gpsimd.dma_start(xd, xs_sb[:])
                        Xb = X_bufs[c % 2]
                        for g in range(NG):
                            src = xd.rearrange(
                                "(s b) (g c) -> g b s c", s=CH, b=BL, g=NG)[g]
                            nc.gpsimd.dma_start(
                                Xb[32 * g:32 * g + BL, :, :], src)
            return run

        for b2 in range(2):
            for k in range(-1, KT):
                proj_pending.append(mk(b2, k))

    def drain_proj(n):
        for _ in range(min(n, len(proj_pending))):
            proj_pending.popleft()()

    # --- prologue ---
    queue_proj(0)
    drain_proj(99)

    # 2 persistent recurrence PSUM tiles (step parities). A closed dummy
    # matmul group sets the has_written bits once; they persist, so the
    # per-step start=False matmuls accumulate onto the DVE-copied xp.
    rec = []
    for par in range(2):
        r = rpool.tile([128, CW], F32, tag=f"rec{par}")
        nc.tensor.matmul(r[:], lhsT=ones1[:],
                         rhs=bias_sb[0:1, 0:CW], start=True, stop=True)
        rec.append(r)

    def emit_xp_copy(t):
        """DVE-copy step t's xp strips into its parity psum tile."""
        Xb = X_bufs[(t // CH) % 2]
        nc.scalar.activation(rec[t % 2][:], Xb[:, t % CH, :],
                             mybir.ActivationFunctionType.Copy)

    emit_xp_copy(0)

    T_cur = None
    for t in range(steps):
        c, j = divmod(t, CH)
        if j == 0 and c + 1 < n_chunks:
            queue_proj(c + 1)

        if t + 1 < steps:
            emit_xp_copy(t + 1)

        r = rec[t % 2]
        if t > 0:
            # 32 matmuls: k-tiles x column-groups (groups stream
            # concurrently in distinct 32-col PE strips), N=256 each.
            for k in range(KT):
                lhsT = T_cur[:, 32 * k:32 * k + BL]
                for g in range(NG):
                    nc.tensor.matmul(
                        r[32 * g:32 * g + BL, :],
                        lhsT=lhsT,
                        rhs=u_sb[:, k, CW * g:CW * (g + 1)],
                        start=False, stop=False, skip_group_check=True,
                        tile_position=(0, 32 * g))

        drain_proj(1)

        # tanh: psum -> H_sb bf16; single wide instructions (fewer
        # instruction/sem overheads beat finer chain splits here)
        h_t = hpool.tile([128, CW], BF16, tag="h")
        nc.scalar.activation(h_t[:], r[:],
                             mybir.ActivationFunctionType.Tanh)

        if t + 1 < steps:
            T_next = tpool.tile([128, CW], BF16, tag="t")
            nc.vector.transpose(T_next[:], h_t[:])
            T_cur = T_next

        # output DMA: out[8t+b, 256g+c] <- H[32g+b, c]
        for g in range(NG):
            nc.sync.dma_start(
                out[BL * t:BL * (t + 1), CW * g:CW * (g + 1)],
                h_t[32 * g:32 * g + BL, :])

        drain_proj(1)


def build_nc(steps=S, enable_asserts=False):
    nc = bacc.Bacc("TRN2", target_bir_lowering=False, debug=False,
                   enable_asserts=enable_asserts)
    xT = nc.dram_tensor("xT", [I, steps * BL], F32R, kind="ExternalInput").ap()
    w = nc.dram_tensor("w", [I, H], F32R, kind="ExternalInput").ap()
    u = nc.dram_tensor("u", [128, KT * H], BF16, kind="ExternalInput").ap()
    bias = nc.dram_tensor("bias", [1, H], F32R, kind="ExternalInput").ap()
    out = nc.dram_tensor("out", [steps * BL, H], BF16, kind="ExternalOutput").ap()
    with tile.TileContext(nc) as tc:
        with ExitStack() as ctx:
            _emit_body(ctx, tc, xT, w, u, bias, out, steps)
    nc.compile()
    return nc


def round_f32r(a):
    """Round fp32 to the FP32R format (11 mantissa bits, RNE, low 12 bits 0)."""
    u = np.ascontiguousarray(a, dtype=np.float32).view(np.uint32)
    u = u + np.uint32(0x7FF) + ((u >> np.uint32(12)) & np.uint32(1))
    u &= np.uint32(0xFFFFF000)
    return u.view(np.float32)


def _bf16(a):
    return np.ascontiguousarray(a).astype(mybir.dt.np(mybir.dt.bfloat16))


def make_in_maps(x, W_ih_f, W_hh_f, b_ih_f, b_hh_f, W_ih_b, W_hh_b, b_ih_b, b_hh_b,
                 steps=S):
    """Build the 8 per-core input dicts. Cores 0-3 fwd, 4-7 bwd."""
    x = np.ascontiguousarray(np.asarray(x, dtype=np.float32)[:, :steps])
    sets = {}
    for d, (Wih, Whh, bi, bh) in (
            ("f", (W_ih_f, W_hh_f, b_ih_f, b_hh_f)),
            ("b", (W_ih_b, W_hh_b, b_ih_b, b_hh_b))):
        u_host = np.ascontiguousarray(np.asarray(Whh).T.astype(np.float32))
        # u_dram[p, k, n] = u_host[256*(p//32) + 32*k + p%32, n]
        u_perm = u_host.reshape(NG, KT, 32, H).transpose(0, 2, 1, 3)
        u_perm = np.ascontiguousarray(u_perm.reshape(128, KT * H))
        sets[d] = (
            round_f32r(np.ascontiguousarray(np.asarray(Wih).T.astype(np.float32))),
            _bf16(u_perm),
            round_f32r((np.asarray(bi) + np.asarray(bh)).astype(np.float32)[None, :]),
        )
    in_maps = []
    for core in range(NCORES):
        d = "f" if core < 4 else "b"
        g = core % 4
        wmat, umat, bsum = sets[d]
        xs = x[BL * g:BL * (g + 1)]
        if d == "b":
            xs = xs[:, ::-1]
        # xT[i, s*BL + b] = xs[b, s, i]
        xT = np.ascontiguousarray(xs.transpose(2, 1, 0).reshape(I, steps * BL))
        in_maps.append({
            "xT": round_f32r(xT),
            "w": wmat,
            "u": umat,
            "bias": bsum,
        })
    return in_maps


def assemble(results, steps=S):
    """results: list of 8 dicts with 'out' [steps*BL, H] bf16 -> [B, steps, 2H]."""
    full = np.empty((B, steps, 2 * H), dtype=np.float32)
    for core in range(NCORES):
        o = np.asarray(results[core]["out"]).astype(np.float32)
        o = o.reshape(steps, BL, H)
        g = core % 4
        if core < 4:
            full[BL * g:BL * (g + 1), :, :H] = o.transpose(1, 0, 2)
        else:
            full[BL * g:BL * (g + 1), :, H:] = o[::-1].transpose(1, 0, 2)
    return full


def kernel(x, W_ih_f, W_hh_f, b_ih_f, b_hh_f, W_ih_b, W_hh_b, b_ih_b, b_hh_b):
    from concourse.bass_utils import run_bass_kernel_spmd
    nc = build_nc(S)
    in_maps = make_in_maps(x, W_ih_f, W_hh_f, b_ih_f, b_hh_f,
                           W_ih_b, W_hh_b, b_ih_b, b_hh_b)
    res = run_bass_kernel_spmd(nc, in_maps, list(range(NCORES))).results
    return assemble(res)


# revision 7
# speedup vs baseline: 1.0881x; 1.0881x over previous
"""Bidirectional tanh-RNN (B=32, S=512, I=H=1024) on 8 Trainium2 NeuronCores.

Sharding: 2 direction groups x 4 cores (cores 0-3 fwd, 4-7 bwd; host
reverses time for bwd). Batch split 4 ways -> BL=8 sequences per core.

Per-core kernel v2 (bf16 recurrence, PE column-tiled):
  - Recurrence h @ W_hh.T runs as 64 matmuls/step: 4 PE column-groups
    (tile_position=(0,32g)) x 8 k-tiles x 2 N-halves of 128. The four
    column groups stream concurrently -> ~2048 PE cycles/step instead of
    8192. Stationary = hT slices [128,8] bf16 (tiny loads); moving =
    W_hhT rows (resident bf16, host-permuted so that the stream-transpose
    block layout of h is consumed directly).
  - PSUM layout: one [128,256] tile per step parity; column-group g owns
    partitions 32g..32g+7, holding h columns 256g..256g+255.
  - tanh: 2 ACT instrs ([128,0:128], [128,128:256]) -> H_sb bf16.
  - hT: 2 DVE 32x32 stream-transposes ([128,128] each) -> T_sb; block
    (a,b_) of T holds h[j, 256a+32b_+i] at [32a+i, 32b_+j], so the
    k-tile-k stationary is simply T[:, 32k:32k+8] given the u row
    permutation kappa(p,k) = 256*(p//32) + 32k + p%32.
  - xp = x @ W_ih.T + bias precomputed in 16-step chunks (f32r, N=512
    matmuls through 2 PSUM banks), staged to strip-layout SBUF tiles via
    one SBUF->SBUF DMA per chunk, DVE-copied [128,256] into the parity
    PSUM one step ahead (has_written bits persist from prologue dummies).
  - Output: h stored bf16, out[8t+b, n] = h_t[b, n]; host converts.
"""

import numpy as np
from contextlib import ExitStack

import concourse.bass as bass
import concourse.mybir as mybir
import concourse.tile as tile
from concourse import bacc

F32 = mybir.dt.float32
F32R = mybir.dt.float32r
BF16 = mybir.dt.bfloat16

B, S, I, H = 32, 512, 1024, 1024
NCORES = 8
BL = 8          # local batch per core
KT = 8          # 128-row contraction tiles over I/H
CH = 16         # steps per projection chunk (M-tile of 128 = 16*8 rows)
NG = 4          # PE column groups
CW = 256        # h columns per group


def _emit_body(ctx: ExitStack, tc: tile.TileContext, xT, w, u, bias, out, steps):
    nc = tc.nc
    n_chunks = steps // CH
    assert steps % CH == 0

    const = ctx.enter_context(tc.tile_pool(name="const", bufs=1))
    xpool = ctx.enter_context(tc.tile_pool(name="xc", bufs=2))
    ppool = ctx.enter_context(tc.tile_pool(name="proj", bufs=1, space="PSUM"))
    rpool = ctx.enter_context(tc.tile_pool(name="rec", bufs=1, space="PSUM"))
    pspool = ctx.enter_context(tc.tile_pool(name="pjs", bufs=2))
    hpool = ctx.enter_context(tc.tile_pool(name="h", bufs=4))
    tpool = ctx.enter_context(tc.tile_pool(name="t", bufs=4))
    dpool = ctx.enter_context(tc.tile_pool(name="dram", bufs=1, space="DRAM"))

    # --- constants / resident weights ---
    w_sb = const.tile([128, KT, H], F32R)
    nc.gpsimd.dma_start(w_sb[:], w.rearrange("(k p) n -> p k n", p=128))
    u_sb = const.tile([128, KT, H], BF16)
    nc.gpsimd.dma_start(u_sb[:], u.rearrange("p (k n) -> p k n", k=KT))
    bias_sb = const.tile([1, H], F32R)
    nc.gpsimd.dma_start(bias_sb[:], bias[:])
    ones1_f = const.tile([1, 128], F32)
    nc.gpsimd.memset(ones1_f[:], 1.0)
    ones1 = const.tile([1, 128], F32R)
    nc.vector.tensor_copy(ones1[:], ones1_f[:])

    # strip-layout xp staging tiles (persistent; garbage rows zeroed once)
    X_bufs = [const.tile([128, CH, CW], F32, name=f"Xb{i}") for i in range(2)]
    for Xb in X_bufs:
        nc.gpsimd.memset(Xb[:], 0.0)

    xT_r = xT.rearrange("(k p) m -> p k m", p=128)
    xp_dram = dpool.tile([steps * BL, H], F32)

    from collections import deque
    proj_pending = deque()

    def queue_proj(c):
        """Queue chunk c's projection matmuls as bubble-filler thunks."""
        xc = xpool.tile([128, KT, CH * BL], F32R, tag="xc")
        nc.gpsimd.dma_start(xc[:], xT_r[:, :, c * CH * BL:(c + 1) * CH * BL])
        xs_sb = pspool.tile([128, H], F32, tag="pjs")
        state = {}

        def mk(b2, k):
            nbs = slice(512 * b2, 512 * (b2 + 1))

            def run():
                if k == -1:
                    p = ppool.tile([128, 512], F32, tag=f"pj{b2}")
                    state[b2] = p
                    nc.tensor.matmul(p[:], lhsT=ones1[:],
                                     rhs=bias_sb[:, nbs],
                                     start=True, stop=False)
                    return
                p = state[b2]
                nc.tensor.matmul(p[:], lhsT=xc[:, k, :],
                                 rhs=w_sb[:, k, nbs],
                                 start=False, stop=(k == KT - 1))
                if k == KT - 1:
                    # bank 0 copy on DVE, bank 1 on ACT: halves the per-chunk
                    # copy spike queued in front of chain-critical tanh (ACT)
                    # and transpose (DVE).
                    if b2 == 0:
                        nc.vector.tensor_copy(xs_sb[:, nbs], p[:])
                    else:
                        nc.scalar.activation(xs_sb[:, nbs], p[:],
                                             mybir.ActivationFunctionType.Copy)
                    if b2 == 1:
                        # stage to DRAM, then strip-relayout back:
                        # X[32g+b, s, c] <- xp_dram[128c + 8s + b, 256g + c]
                        xd = xp_dram[128 * c:128 * (c + 1), :]
                        nc.gpsimd.dma_start(xd, xs_sb[:])
                        Xb = X_bufs[c % 2]
                        for g in range(NG):
                            src = xd.rearrange(
                                "(s b) (g c) -> g b s c", s=CH, b=BL, g=NG)[g]
                            nc.gpsimd.dma_start(
                                Xb[32 * g:32 * g + BL, :, :], src)
            return run

        for b2 in range(2):
            for k in range(-1, KT):
                proj_pending.append(mk(b2, k))

    def drain_proj(n):
        for _ in range(min(n, len(proj_pending))):
            proj_pending.popleft()()

    # --- prologue ---
    queue_proj(0)
    drain_proj(99)

    # 2 persistent recurrence PSUM tiles (step parities). A closed dummy
    # matmul group sets the has_written bits once; they persist, so the
    # per-step start=False matmuls accumulate onto the DVE-copied xp.
    rec = []
    for par in range(3):
        r = rpool.tile([128, CW], F32, tag=f"rec{par}")
        nc.tensor.matmul(r[:], lhsT=ones1[:],
                         rhs=bias_sb[0:1, 0:CW], start=True, stop=True)
        rec.append(r)

    def emit_xp_copy(t):
        """DVE-copy step t's xp strips into its parity psum tile."""
        Xb = X_bufs[(t // CH) % 2]
        nc.scalar.activation(rec[t % 3][:], Xb[:, t % CH, :],
                             mybir.ActivationFunctionType.Copy)

    emit_xp_copy(0)

    T_cur = None
    for t in range(steps):
        c, j = divmod(t, CH)
        if j == 0 and c + 1 < n_chunks:
            queue_proj(c + 1)

        if t + 1 < steps:
            emit_xp_copy(t + 1)

        r = rec[t % 3]
        if t > 0:
            # 32 matmuls: k-tiles x column-groups (groups stream
            # concurrently in distinct 32-col PE strips), N=256 each.
            for k in range(KT):
                lhsT = T_cur[:, 32 * k:32 * k + BL]
                for g in range(NG):
                    nc.tensor.matmul(
                        r[32 * g:32 * g + BL, :],
                        lhsT=lhsT,
                        rhs=u_sb[:, k, CW * g:CW * (g + 1)],
                        start=False, stop=False, skip_group_check=True,
                        tile_position=(0, 32 * g))

        drain_proj(1)

        # tanh: psum -> H_sb bf16; single wide instructions (fewer
        # instruction/sem overheads beat finer chain splits here)
        h_t = hpool.tile([128, CW], BF16, tag="h")
        nc.scalar.activation(h_t[:], r[:],
                             mybir.ActivationFunctionType.Tanh)

        if t + 1 < steps:
            T_next = tpool.tile([128, CW], BF16, tag="t")
            nc.vector.transpose(T_next[:], h_t[:])
            T_cur = T_next

        # output DMA: out[8t+b, 256g+c] <- H[32g+b, c]
        for g in range(NG):
            nc.sync.dma_start(
                out[BL * t:BL * (t + 1), CW * g:CW * (g + 1)],
                h_t[32 * g:32 * g + BL, :])

        drain_proj(1)


def build_nc(steps=S, enable_asserts=False):
    nc = bacc.Bacc("TRN2", target_bir_lowering=False, debug=False,
                   enable_asserts=enable_asserts)
    xT = nc.dram_tensor("xT", [I, steps * BL], F32R, kind="ExternalInput").ap()
    w = nc.dram_tensor("w", [I, H], F32R, kind="ExternalInput").ap()
    u = nc.dram_tensor("u", [128, KT * H], BF16, kind="ExternalInput").ap()
    bias = nc.dram_tensor("bias", [1, H], F32R, kind="ExternalInput").ap()
    out = nc.dram_tensor("out", [steps * BL, H], BF16, kind="ExternalOutput").ap()
    with tile.TileContext(nc) as tc:
        with ExitStack() as ctx:
            _emit_body(ctx, tc, xT, w, u, bias, out, steps)
    nc.compile()
    return nc


def round_f32r(a):
    """Round fp32 to the FP32R format (11 mantissa bits, RNE, low 12 bits 0)."""
    u = np.ascontiguousarray(a, dtype=np.float32).view(np.uint32)
    u = u + np.uint32(0x7FF) + ((u >> np.uint32(12)) & np.uint32(1))
    u &= np.uint32(0xFFFFF000)
    return u.view(np.float32)


def _bf16(a):
    return np.ascontiguousarray(a).astype(mybir.dt.np(mybir.dt.bfloat16))


def make_in_maps(x, W_ih_f, W_hh_f, b_ih_f, b_hh_f, W_ih_b, W_hh_b, b_ih_b, b_hh_b,
                 steps=S):
    """Build the 8 per-core input dicts. Cores 0-3 fwd, 4-7 bwd."""
    x = np.ascontiguousarray(np.asarray(x, dtype=np.float32)[:, :steps])
    sets = {}
    for d, (Wih, Whh, bi, bh) in (
            ("f", (W_ih_f, W_hh_f, b_ih_f, b_hh_f)),
            ("b", (W_ih_b, W_hh_b, b_ih_b, b_hh_b))):
        u_host = np.ascontiguousarray(np.asarray(Whh).T.astype(np.float32))
        # u_dram[p, k, n] = u_host[256*(p//32) + 32*k + p%32, n]
        u_perm = u_host.reshape(NG, KT, 32, H).transpose(0, 2, 1, 3)
        u_perm = np.ascontiguousarray(u_perm.reshape(128, KT * H))
        sets[d] = (
            round_f32r(np.ascontiguousarray(np.asarray(Wih).T.astype(np.float32))),
            _bf16(u_perm),
            round_f32r((np.asarray(bi) + np.asarray(bh)).astype(np.float32)[None, :]),
        )
    in_maps = []
    for core in range(NCORES):
        d = "f" if core < 4 else "b"
        g = core % 4
        wmat, umat, bsum = sets[d]
        xs = x[BL * g:BL * (g + 1)]
        if d == "b":
            xs = xs[:, ::-1]
        # xT[i, s*BL + b] = xs[b, s, i]
        xT = np.ascontiguousarray(xs.transpose(2, 1, 0).reshape(I, steps * BL))
        in_maps.append({
            "xT": round_f32r(xT),
            "w": wmat,
            "u": umat,
            "bias": bsum,
        })
    return in_maps


def assemble(results, steps=S):
    """results: list of 8 dicts with 'out' [steps*BL, H] bf16 -> [B, steps, 2H]."""
    full = np.empty((B, steps, 2 * H), dtype=np.float32)
    for core in range(NCORES):
        o = np.asarray(results[core]["out"]).astype(np.float32)
        o = o.reshape(steps, BL, H)
        g = core % 4
        if core < 4:
            full[BL * g:BL * (g + 1), :, :H] = o.transpose(1, 0, 2)
        else:
            full[BL * g:BL * (g + 1), :, H:] = o[::-1].transpose(1, 0, 2)
    return full


def kernel(x, W_ih_f, W_hh_f, b_ih_f, b_hh_f, W_ih_b, W_hh_b, b_ih_b, b_hh_b):
    from concourse.bass_utils import run_bass_kernel_spmd
    nc = build_nc(S)
    in_maps = make_in_maps(x, W_ih_f, W_hh_f, b_ih_f, b_hh_f,
                           W_ih_b, W_hh_b, b_ih_b, b_hh_b)
    res = run_bass_kernel_spmd(nc, in_maps, list(range(NCORES))).results
    return assemble(res)
